# revision 1
# baseline (speedup 1.0000x reference)
"""GAT (2-layer PyG GATConv, eval) on 8 Trainium2 NeuronCores.

Sharding: nodes range-partitioned (NLOC=12800/core); each core owns edges whose
dst is in its range, grouped by (128-dst window, src quadrant) into static
640-slot groups. Per edge: [h|alpha_src] rows come from dma_gather (int16 idx
into 25600-row sub-tables), alpha_dst from a 32B-payload dma_gather on a local
table. Softmax uses the shift-invariant no-max form (|e| < ~25, fp32 exp safe):
w = exp(leaky_relu(as+ad)); out = sum(w h)/sum(w). Segment reduction is a PE
matmul with a one-hot(dst_rel) stationary matrix accumulating dst-major
[128, C] PSUM per window — no scatter instructions (HW scatter-add races on
duplicate indices). One transposed-fp16 AllGather links the layers.
"""
import numpy as np
import ml_dtypes

N = 100000
E = 1600000
NF = 256
HEADS, NHID = 8, 8
NH = HEADS * NHID          # 64
NCLASS = 40
NLOC = 12800               # nodes per core
NW = 100                   # 128-dst windows per core
NQ = 4                     # src quadrants
QS = 25600                 # sub-table rows per quadrant
SQ = 640                   # edge slots per (window, quadrant) group
CPG = SQ // 128            # 5 chunks per group
CPW = CPG * NQ             # 20 chunks per window
NSLOT = NW * NQ * SQ       # 256000 slots per core
NTOT = 102400
NCHK = 100352 // 128       # 784 node chunks in the global pass
ACC_EPS = 1e-16

_CACHE = {}


def _host_prep(x, edge_index, W1, a1_src, a1_dst, b1, W2, a2_src, a2_dst, b2):
    src = np.asarray(edge_index[0], dtype=np.int64)
    dst = np.asarray(edge_index[1], dtype=np.int64)

    core = dst // NLOC
    dloc = dst - core * NLOC
    win = dloc >> 7
    q = src // QS
    gid = ((core * NW) + win) * NQ + q
    order = np.argsort(gid, kind="stable")
    gsz = np.bincount(gid, minlength=8 * NW * NQ)
    assert gsz.max() <= SQ, f"group overflow: {gsz.max()} > {SQ}"
    starts = np.zeros_like(gsz)
    starts[1:] = np.cumsum(gsz)[:-1]
    g_sorted = gid[order]
    rank = np.arange(E) - starts[g_sorted]
    slot_global = g_sorted * SQ + rank
    cc = slot_global // NSLOT
    sc = slot_global - cc * NSLOT

    hidx = np.zeros((8, NSLOT), np.int16)          # pad -> row 0 of sub-table
    drel = np.full((8, NSLOT), 128.0, np.float32)  # pad -> out-of-window
    hidx[cc, sc] = (src[order] - q[order] * QS).astype(np.int16)
    drel[cc, sc] = (dloc[order] & 127).astype(np.float32)

    # dma_gather idx wrap: token s -> [s%16, s//16], replicated into all eight
    # 16-partition blocks (Q7 cpu pairs for the 4 SWDGE queues).
    slots = np.arange(NSLOT)
    hw = np.zeros((8, 128, NSLOT // 16), np.int16)
    hw[:, slots % 16, slots // 16] = hidx
    for r in range(1, 8):
        hw[:, 16 * r:16 * (r + 1)] = hw[:, :16]

    # dst_rel, chunk-major: token s -> [s%128, s//128]
    dw = np.zeros((8, 128, NSLOT // 128), np.float32)
    dw[:, slots % 128, slots // 128] = drel
    dw = dw.astype(ml_dtypes.bfloat16)

    # fold attention vectors into the node-pass weights
    W1 = np.asarray(W1, np.float32)
    v_s1 = np.einsum("chk,hk->ch", W1.reshape(NF, HEADS, NHID),
                     np.asarray(a1_src, np.float32))
    v_d1 = np.einsum("chk,hk->ch", W1.reshape(NF, HEADS, NHID),
                     np.asarray(a1_dst, np.float32))
    W1e = np.concatenate([W1, v_s1, v_d1], axis=1).reshape(2, 128, 80)
    W1e = W1e.astype(np.float16)

    W2 = np.asarray(W2, np.float32)
    v_s2 = W2 @ np.asarray(a2_src, np.float32)[0]
    v_d2 = W2 @ np.asarray(a2_dst, np.float32)[0]
    W2e = np.concatenate([W2, v_s2[:, None]], axis=1).astype(np.float16)

    xp = np.zeros((NTOT, NF), np.float32)
    xp[:N] = np.asarray(x, np.float32)
    xT = np.ascontiguousarray(xp[:100352].T).astype(np.float16)
    xT = xT.reshape(2, 128, 100352)

    per_core = []
    for c in range(8):
        xloc = np.ascontiguousarray(xp[c * NLOC:(c + 1) * NLOC].T)
        per_core.append({
            "xT": xT,
            "xTloc": xloc.astype(np.float16).reshape(2, 128, NLOC),
            "W1e": W1e,
            "W2e": W2e,
            "vd2": v_d2[None, :].astype(np.float16),
            "b1": np.asarray(b1, np.float32)[None, :],
            "b2": np.asarray(b2, np.float32)[None, :],
            "hidx": hw[c],
            "drel": dw[c],
        })
    return per_core


def _dma_gather_small(g, out_ap, in_ap, idxs_ap, num_idxs, elem_size,
                      elem_step, queue_num=0):
    """dma_gather with a <256B payload. Only the row STRIDE must be a 256B
    multiple on the Q7 side; bass's elem_size%256 assert is over-conservative
    for the non-transpose path, so build the instruction directly."""
    import concourse.mybir as mybir
    stride_bytes = elem_step * mybir.dt.size(in_ap.dtype)
    assert stride_bytes % 256 == 0
    _in_ap = g.lower_ap_dma(in_ap, for_custom_bir_dma=True)
    _idxs_ap = g.lower_ap(idxs_ap)
    _out_ap = g.lower_ap(out_ap)
    return g.add_instruction(mybir.InstDMAGatherAnt(
        name=g.bass.get_next_instruction_name(),
        ins=[*_in_ap, _idxs_ap, g.lower_val_access(g.to_reg(num_idxs))],
        outs=[_out_ap],
        transpose=False,
        num_idxs=num_idxs,
        elem_size=elem_size,
        stride_bytes_256=stride_bytes // 256,
        gen_mode=0,
        single_packet=True,
        queue_num=queue_num,
        sbuf_tokens_per_rank=0,
        sbuf_free_dim_per_rank=0,
        sbuf_free_dim_pad_per_rank=0,
        sbuf_byte_offset=0,
    ))


def _build_nc():
    import concourse.bass as bass
    import concourse.bacc as bacc
    import concourse.mybir as mybir
    import concourse.tile as tile
    from concourse.library_config import mlp
    from concourse.masks import make_identity

    f32, f16, bf16, i16 = (mybir.dt.float32, mybir.dt.float16,
                           mybir.dt.bfloat16, mybir.dt.int16)
    AF = mybir.ActivationFunctionType
    OP = mybir.AluOpType

    nc = bacc.Bacc("TRN2", target_bir_lowering=False, debug=False,
                   num_devices=8, num_swdge_queues=4)

    xT = nc.dram_tensor("xT", [2, 128, 100352], f16, kind="ExternalInput")
    xTloc = nc.dram_tensor("xTloc", [2, 128, NLOC], f16, kind="ExternalInput")
    W1e = nc.dram_tensor("W1e", [2, 128, 80], f16, kind="ExternalInput")
    W2e = nc.dram_tensor("W2e", [64, 41], f16, kind="ExternalInput")
    vd2 = nc.dram_tensor("vd2", [1, 64], f16, kind="ExternalInput")
    b1 = nc.dram_tensor("b1", [1, 64], f32, kind="ExternalInput")
    b2 = nc.dram_tensor("b2", [1, 40], f32, kind="ExternalInput")
    hidx = nc.dram_tensor("hidx", [128, NSLOT // 16], i16, kind="ExternalInput")
    drel = nc.dram_tensor("drel", [128, NSLOT // 128], bf16, kind="ExternalInput")
    out = nc.dram_tensor("out", [NLOC, 40], f32, kind="ExternalOutput")

    tab1 = nc.dram_tensor("tab1", [NTOT, 128], f16)          # [h1|as1|pad]
    taba1 = nc.dram_tensor("taba1", [NLOC, 8], bf16)         # ad1 per local dst
    tab2 = nc.dram_tensor("tab2", [NTOT, 64], f32)           # [h2|as2|pad]
    taba2 = nc.dram_tensor("taba2", [NLOC, 8], bf16)         # ad2 in all cols
    agi = nc.dram_tensor("agi", [64, NLOC], f16)
    ago = nc.dram_tensor("ago", [512, NLOC], f16, addr_space="Shared")

    def BC(ap, dims):
        return bass.AP(ap.tensor, ap.offset, dims)

    with tile.TileContext(nc) as tc:
        with tc.tile_pool(name="const", bufs=1) as pc:
            nc.gpsimd.load_library(mlp)

            hidx_sb = pc.tile([128, NSLOT // 16], i16)
            drel_sb = pc.tile([128, NSLOT // 128], bf16)
            nc.sync.dma_start(hidx_sb[:], hidx[:])
            nc.sync.dma_start(drel_sb[:], drel[:])
            w1_sb = pc.tile([128, 2, 80], f16)
            nc.sync.dma_start(w1_sb[:], W1e[:].rearrange("k p n -> p k n"))
            w2_sb = pc.tile([64, 41], f16)
            nc.sync.dma_start(w2_sb[:], W2e[:])

            iota_i = pc.tile([128, 128], i16)
            nc.gpsimd.iota(iota_i[:], pattern=[[1, 128]], base=0,
                           channel_multiplier=0)
            iota_sb = pc.tile([128, 128], bf16)
            nc.vector.tensor_copy(out=iota_sb[:], in_=iota_i[:])

            ident = pc.tile([128, 128], f16)
            make_identity(nc, ident[:])
            identb = pc.tile([128, 128], bf16)
            make_identity(nc, identb[:])

            ones32 = pc.tile([1, 128], f32)
            nc.vector.memset(ones32[:], 1.0)
            ones16 = pc.tile([1, 128], f16)
            nc.vector.memset(ones16[:], 1.0)

            b1r = pc.tile([128, 64], f32)
            b2r = pc.tile([128, 40], f32)
            vd2r = pc.tile([128, 64], f32)
            with tc.tile_pool(name="pini", bufs=2, space="PSUM") as ppi:
                for row_d, width, rdt, dest in (
                        (b1, 64, f32, b1r), (b2, 40, f32, b2r),
                        (vd2, 64, f16, vd2r)):
                    t = pc.tile([1, width], rdt, tag=f"rrow{width}{rdt}")
                    nc.sync.dma_start(t[:], row_d[:])
                    ps = ppi.tile([128, width], f32, tag="rep")
                    lhs = ones32 if rdt == f32 else ones16
                    nc.tensor.matmul(ps[:], lhsT=lhs[:], rhs=t[:],
                                     start=True, stop=True)
                    nc.vector.tensor_copy(out=dest[:], in_=ps[:])

            # ---------- phase A: global node pass -> tab1 ----------
            with (tc.tile_pool(name="pa", bufs=3) as pa,
                  tc.tile_pool(name="ppa", bufs=2, space="PSUM") as ppa):
                for i0 in range(0, NCHK, 4):
                    nb = min(4, NCHK - i0)
                    xt = pa.tile([128, 2, 512], f16, tag="xt")
                    for k in range(2):
                        nc.sync.dma_start(xt[:, k, :nb * 128],
                                          xT[k, :, i0 * 128:(i0 + nb) * 128])
                    row = pa.tile([128, 4, 128], f16, tag="row")
                    for j in range(nb):
                        ps = ppa.tile([128, 80], f32, tag="np1")
                        for k in range(2):
                            nc.tensor.matmul(
                                ps[:], lhsT=xt[:, k, j * 128:(j + 1) * 128],
                                rhs=w1_sb[:, k, :], start=(k == 0),
                                stop=(k == 1))
                        nc.vector.tensor_copy(out=row[:, j, :72],
                                              in_=ps[:, :72])
                        nc.vector.memset(row[:, j, 72:], 0.0)
                    nc.sync.dma_start(
                        tab1[i0 * 128:(i0 + nb) * 128, :].rearrange(
                            "(a b) c -> b a c", b=128),
                        row[:, :nb, :])

                # ---------- phase A2: local pass -> taba1 (ad1) ----------
                for i0 in range(0, NW, 4):
                    xt = pa.tile([128, 2, 512], f16, tag="xt")
                    for k in range(2):
                        nc.sync.dma_start(
                            xt[:, k, :], xTloc[k, :, i0 * 128:(i0 + 4) * 128])
                    ad = pa.tile([128, 4, 8], bf16, tag="ad")
                    for j in range(4):
                        ps = ppa.tile([128, 80], f32, tag="np1")
                        for k in range(2):
                            nc.tensor.matmul(
                                ps[:], lhsT=xt[:, k, j * 128:(j + 1) * 128],
                                rhs=w1_sb[:, k, :], start=(k == 0),
                                stop=(k == 1))
                        nc.vector.tensor_copy(out=ad[:, j, :], in_=ps[:, 72:80])
                    nc.sync.dma_start(
                        taba1[i0 * 128:(i0 + 4) * 128, :].rearrange(
                            "(a b) c -> b a c", b=128),
                        ad[:])

            # ---------- phase B: layer-1 edge pass ----------
            with (tc.tile_pool(name="pb", bufs=3) as pb,
                  tc.tile_pool(name="ppb", bufs=2, space="PSUM") as ppb):
                for w in range(NW):
                    ht = pb.tile([128, CPW, 128], f16, tag="ht")
                    for qi in range(NQ):
                        g = w * NQ + qi
                        nc.gpsimd.dma_gather(
                            ht[:, qi * CPG:(qi + 1) * CPG, :],
                            tab1[qi * QS:(qi + 1) * QS, :],
                            hidx_sb[:, g * (SQ // 16):(g + 1) * (SQ // 16)],
                            SQ, SQ, 128, queue_num=qi)
                    aw1 = pb.tile([128, 8], bf16, tag="aw1")
                    nc.sync.dma_start(aw1[:], taba1[w * 128:(w + 1) * 128, :])

                    dr = drel_sb[:, w * CPW:(w + 1) * CPW]
                    oh = pb.tile([128, CPW, 128], bf16, tag="oh")
                    nc.vector.tensor_tensor(
                        out=oh[:],
                        in0=BC(dr, [dr.ap[0], dr.ap[1], [0, 128]]),
                        in1=BC(iota_sb[:], [iota_sb[:].ap[0], [0, CPW], [1, 128]]),
                        op=OP.is_equal)

                    # alpha_dst expansion: at[:, k, :] = oh[:, k, :] @ aw1
                    at = pb.tile([128, CPW, 8], f32, tag="at")
                    for k in range(CPW):
                        ohp = ppb.tile([128, 128], bf16, tag="ohT")
                        nc.tensor.transpose(out=ohp[:], in_=oh[:, k, :],
                                            identity=identb[:])
                        ohs = pb.tile([128, 128], bf16, tag="ohs")
                        nc.scalar.copy(out=ohs[:], in_=ohp[:])
                        adp = ppb.tile([128, 8], f32, tag="adp")
                        nc.tensor.matmul(adp[:], lhsT=ohs[:], rhs=aw1[:],
                                         start=True, stop=True)
                        nc.vector.tensor_copy(out=at[:, k, :], in_=adp[:])

                    e1 = pb.tile([128, CPW, 8], f32, tag="e1")
                    nc.vector.tensor_tensor(out=e1[:], in0=ht[:, :, 64:72],
                                            in1=at[:], op=OP.add)
                    ls = pb.tile([128, CPW, 8], f32, tag="ls")
                    nc.vector.tensor_scalar_mul(ls[:], e1[:], 0.2)
                    lr = pb.tile([128, CPW, 8], f32, tag="lr")
                    nc.vector.tensor_tensor(out=lr[:], in0=e1[:], in1=ls[:],
                                            op=OP.max)
                    wg = pb.tile([128, CPW, 8], f32, tag="wg")
                    nc.scalar.activation(out=wg[:], in_=lr[:], func=AF.Exp)

                    msg = pb.tile([128, CPW, 72], bf16, tag="msg")
                    mfull = msg[:]
                    hfull = ht[:]
                    wfull = wg[:]
                    nc.vector.tensor_tensor(
                        out=BC(mfull, [mfull.ap[0], [72, CPW], [8, 8], [1, 8]]),
                        in0=BC(hfull, [hfull.ap[0], [128, CPW], [8, 8], [1, 8]]),
                        in1=BC(wfull, [wfull.ap[0], [8, CPW], [1, 8], [0, 8]]),
                        op=OP.mult)
                    nc.vector.tensor_copy(out=msg[:, :, 64:72], in_=wg[:])

                    ps = ppb.tile([128, 72], f32, tag="agg")
                    for k in range(CPW):
                        nc.tensor.matmul(ps[:], lhsT=oh[:, k, :],
                                         rhs=msg[:, k, :], start=(k == 0),
                                         stop=(k == CPW - 1))

                    den = pb.tile([128, 8], f32, tag="den")
                    nc.vector.tensor_scalar_add(den[:], ps[:, 64:72], ACC_EPS)
                    rec = pb.tile([128, 8], f32, tag="rec")
                    nc.vector.reciprocal(rec[:], den[:])
                    o1 = pb.tile([128, 64], f32, tag="o1")
                    pnum = ps[:, 0:64]
                    rfull = rec[:]
                    nc.vector.tensor_tensor(
                        out=BC(o1[:], [o1[:].ap[0], [8, 8], [1, 8]]),
                        in0=BC(pnum, [pnum.ap[0], [8, 8], [1, 8]]),
                        in1=BC(rfull, [rfull.ap[0], [1, 8], [0, 8]]),
                        op=OP.mult)
                    o1b = pb.tile([128, 64], f32, tag="o1b")
                    nc.vector.tensor_tensor(out=o1b[:], in0=o1[:], in1=b1r[:],
                                            op=OP.add)
                    # elu = relu(x) + exp(min(x,0)) - 1
                    t1 = pb.tile([128, 64], f32, tag="t1")
                    nc.vector.tensor_scalar_min(t1[:], o1b[:], 0.0)
                    t2 = pb.tile([128, 64], f32, tag="t2")
                    nc.scalar.activation(out=t2[:], in_=t1[:], func=AF.Exp)
                    t3 = pb.tile([128, 64], f32, tag="t3")
                    nc.vector.tensor_scalar_max(t3[:], o1b[:], 0.0)
                    t4 = pb.tile([128, 64], f32, tag="t4")
                    nc.vector.tensor_tensor(out=t4[:], in0=t2[:], in1=t3[:],
                                            op=OP.add)
                    hl = pb.tile([128, 64], f32, tag="hl")
                    nc.vector.tensor_scalar_add(hl[:], t4[:], -1.0)

                    # ad2 for layer 2
                    t5 = pb.tile([128, 64], f32, tag="t5")
                    nc.vector.tensor_tensor(out=t5[:], in0=hl[:], in1=vd2r[:],
                                            op=OP.mult)
                    ad2 = pb.tile([128, 1], f32, tag="ad2")
                    nc.vector.tensor_reduce(ad2[:], t5[:],
                                            axis=mybir.AxisListType.X,
                                            op=OP.add)
                    ad2b = pb.tile([128, 8], bf16, tag="ad2b")
                    nc.vector.tensor_copy(out=ad2b[:],
                                          in_=ad2[:].to_broadcast([128, 8]))
                    nc.sync.dma_start(taba2[w * 128:(w + 1) * 128, :], ad2b[:])

                    hl16 = pb.tile([128, 64], f16, tag="hl16")
                    nc.vector.tensor_copy(out=hl16[:], in_=hl[:])
                    pst = ppb.tile([64, 128], f16, tag="tr")
                    nc.tensor.transpose(out=pst[:], in_=hl16[:], identity=ident[:])
                    hlT = pb.tile([64, 128], f16, tag="hlT")
                    nc.vector.tensor_copy(out=hlT[:], in_=pst[:])
                    nc.sync.dma_start(agi[:, w * 128:(w + 1) * 128], hlT[:])

            # ---------- AllGather ----------
            nc.gpsimd.collective_compute(
                "AllGather", OP.bypass, ins=[agi[:]], outs=[ago[:]],
                replica_groups=[list(range(8))])

            # ---------- phase C: layer-2 node pass -> tab2 ----------
            with (tc.tile_pool(name="pcn", bufs=3) as pn,
                  tc.tile_pool(name="ppc", bufs=2, space="PSUM") as ppc):
                for i0 in range(0, NTOT // 128, 4):
                    cn = (i0 * 128) // NLOC
                    lo = (i0 * 128) % NLOC
                    hT = pn.tile([64, 512], f16, tag="hT")
                    nc.sync.dma_start(hT[:],
                                      ago[cn * 64:(cn + 1) * 64, lo:lo + 512])
                    r2 = pn.tile([128, 4, 41], f32, tag="r2")
                    for j in range(4):
                        ps2 = ppc.tile([128, 41], f32, tag="np2")
                        nc.tensor.matmul(ps2[:],
                                         lhsT=hT[:, j * 128:(j + 1) * 128],
                                         rhs=w2_sb[:], start=True, stop=True)
                        nc.vector.tensor_copy(out=r2[:, j, :], in_=ps2[:])
                    nc.sync.dma_start(
                        tab2[i0 * 128:(i0 + 4) * 128, 0:41].rearrange(
                            "(a b) c -> b a c", b=128),
                        r2[:])

            # ---------- phase D: layer-2 edge pass -> out ----------
            with (tc.tile_pool(name="pd", bufs=3) as pd,
                  tc.tile_pool(name="ppd", bufs=2, space="PSUM") as ppd):
                for w in range(NW):
                    g2 = pd.tile([128, CPW, 64], f32, tag="g2")
                    for qi in range(NQ):
                        g = w * NQ + qi
                        nc.gpsimd.dma_gather(
                            g2[:, qi * CPG:(qi + 1) * CPG, :],
                            tab2[qi * QS:(qi + 1) * QS, :],
                            hidx_sb[:, g * (SQ // 16):(g + 1) * (SQ // 16)],
                            SQ, SQ, 64, queue_num=qi)
                    aw2 = pd.tile([128, 8], bf16, tag="aw2")
                    nc.sync.dma_start(aw2[:], taba2[w * 128:(w + 1) * 128, :])

                    dr = drel_sb[:, w * CPW:(w + 1) * CPW]
                    oh = pd.tile([128, CPW, 128], bf16, tag="oh2")
                    nc.vector.tensor_tensor(
                        out=oh[:],
                        in0=BC(dr, [dr.ap[0], dr.ap[1], [0, 128]]),
                        in1=BC(iota_sb[:], [iota_sb[:].ap[0], [0, CPW], [1, 128]]),
                        op=OP.is_equal)

                    a2 = pd.tile([128, CPW, 1], f32, tag="a2")
                    for k in range(CPW):
                        ohp = ppd.tile([128, 128], bf16, tag="ohT2")
                        nc.tensor.transpose(out=ohp[:], in_=oh[:, k, :],
                                            identity=identb[:])
                        ohs = pd.tile([128, 128], bf16, tag="ohs2")
                        nc.scalar.copy(out=ohs[:], in_=ohp[:])
                        adp = ppd.tile([128, 1], f32, tag="adp2")
                        nc.tensor.matmul(adp[:], lhsT=ohs[:], rhs=aw2[:, 0:1],
                                         start=True, stop=True)
                        nc.vector.tensor_copy(out=a2[:, k, :], in_=adp[:])

                    e2 = pd.tile([128, CPW, 1], f32, tag="e2")
                    nc.vector.tensor_tensor(out=e2[:], in0=g2[:, :, 40:41],
                                            in1=a2[:], op=OP.add)
                    ls2 = pd.tile([128, CPW, 1], f32, tag="ls2")
                    nc.vector.tensor_scalar_mul(ls2[:], e2[:], 0.2)
                    lr2 = pd.tile([128, CPW, 1], f32, tag="lr2")
                    nc.vector.tensor_tensor(out=lr2[:], in0=e2[:], in1=ls2[:],
                                            op=OP.max)
                    wg2 = pd.tile([128, CPW, 1], f32, tag="wg2")
                    nc.scalar.activation(out=wg2[:], in_=lr2[:], func=AF.Exp)

                    m2 = pd.tile([128, CPW, 41], bf16, tag="m2")
                    m2f = m2[:]
                    g2f = g2[:]
                    w2f = wg2[:]
                    nc.vector.tensor_tensor(
                        out=BC(m2f, [m2f.ap[0], [41, CPW], [1, 40]]),
                        in0=BC(g2f, [g2f.ap[0], [64, CPW], [1, 40]]),
                        in1=BC(w2f, [w2f.ap[0], [1, CPW], [0, 40]]),
                        op=OP.mult)
                    nc.vector.tensor_copy(out=m2[:, :, 40:41], in_=wg2[:])

                    ps = ppd.tile([128, 41], f32, tag="agg2")
                    for k in range(CPW):
                        nc.tensor.matmul(ps[:], lhsT=oh[:, k, :],
                                         rhs=m2[:, k, :], start=(k == 0),
                                         stop=(k == CPW - 1))

                    den2 = pd.tile([128, 1], f32, tag="den2")
                    nc.vector.tensor_scalar_add(den2[:], ps[:, 40:41], ACC_EPS)
                    rec2 = pd.tile([128, 1], f32, tag="rec2")
                    nc.vector.reciprocal(rec2[:], den2[:])
                    o2 = pd.tile([128, 40], f32, tag="o2")
                    nc.vector.tensor_tensor(out=o2[:], in0=ps[:, 0:40],
                                            in1=rec2[:].to_broadcast([128, 40]),
                                            op=OP.mult)
                    o2b = pd.tile([128, 40], f32, tag="o2b")
                    nc.vector.tensor_tensor(out=o2b[:], in0=o2[:], in1=b2r[:],
                                            op=OP.add)
                    nc.sync.dma_start(out[w * 128:(w + 1) * 128, :], o2b[:])

    nc.finalize()
    return nc


def kernel(**inputs):
    per_core = _host_prep(**inputs)
    if "nc" not in _CACHE:
        _CACHE["nc"] = _build_nc()
    nc = _CACHE["nc"]
    from concourse.bass_utils import run_bass_kernel_spmd
    res = run_bass_kernel_spmd(nc, per_core, list(range(8)))
    full = np.concatenate([res.results[c]["out"] for c in range(8)], axis=0)
    return np.ascontiguousarray(full[:N]).astype(np.float32)



# revision 37
# speedup vs baseline: 2.3219x; 2.3219x over previous
"""GAT (2-layer PyG GATConv, eval) on 8 Trainium2 NeuronCores.

Sharding: nodes range-partitioned (NLOC=12800/core); core c owns edges whose
dst is in its range. Both layers' node tables are computed SHARDED (each core
transforms only its own 12800-node block) and replicated by one AllGather per
layer; each AllGather is fully overlapped by the a_dst gather burst for the
next edge pass, which reads only the local block.

Slot layout per core: superblock (10 windows) -> quadrant -> window, with
per-(window,quadrant) STATIC capacities = max edge count over the 8 cores
(SPMD: one module runs on all cores; only tensor contents differ) — ~10%
slot padding vs 28% for fixed-size groups. Every dma_gather call carries at
most 1024 indices (the SWDGE ucode's ring limit on real hardware; larger
calls hard-crash the device), rotated across the 4 SWDGE queues.

Per layer, per edge slot: a 144B/82B payload gather pulls [h|a_src] rows
(256B-stride tables, int16 idx into 25600-row quadrants); a 16B/4B gather
pulls a_dst. Softmax is the shift-invariant no-max form
(w = exp(leakyrelu(as+ad)), |e| < ~25 so fp32 exp is safe). Segment reduction
is a PE matmul whose stationary matrix is a transposed one-hot built by DVE
is_equal in the 2x-mode layout (both operands packed 2-byte, last-dim stride
1); chunk slices address it with a strided free dim, boundary chunks carry a
masked drel column per touching window. Messages msg = h*w use an
Act-expanded weight tile so the DVE mult also runs in 2x mode. Softmax
normalize + ELU + the r2 = hlT @ [W2|a2_src|a2_dst] projection are batched
per superblock; tables are written node-permuted (row = p*100 + j within
each core block) so table writes coalesce to one descriptor per partition.
"""
import numpy as np
import ml_dtypes

N = 100000
E = 1600000
NF = 256
HEADS, NHID = 8, 8
NH = HEADS * NHID          # 64
NCLASS = 40
NLOC = 12800               # nodes per core
NW = 100                   # 128-dst windows per core
NQ = 4                     # src table quadrants
QS = 25600                 # rows per quadrant
NWSB = 10                  # windows per superblock
NSB = NW // NWSB           # 10 superblocks
NTOT = 102400
NCHK_A = 784               # 100352/128 global node chunks
ACC_EPS = 1e-16

_CACHE = {}


def _ceil128(x):
    return (x + 127) & ~127


def _host_prep(x, edge_index, W1, a1_src, a1_dst, b1, W2, a2_src, a2_dst, b2):
    src = np.asarray(edge_index[0], dtype=np.int64)
    dst = np.asarray(edge_index[1], dtype=np.int64)

    # table-row permutation: node n -> row  c*NLOC + (l%128)*NW + l//128
    def rowperm(n):
        c = n // NLOC
        l = n - c * NLOC
        return c * NLOC + (l % 128) * NW + l // 128

    srow = rowperm(src)
    sq = srow // QS
    sidx = (srow - sq * QS).astype(np.int16)

    core = dst // NLOC
    dloc = dst - core * NLOC
    w_e = (dloc >> 7).astype(np.int64)
    dr_e = (dloc & 127).astype(np.int64)
    adidx = (dr_e * NW + w_e).astype(np.int16)
    sb_e = w_e // NWSB

    # static capacities: max over cores per (window, quadrant)
    gkey = (core * NW + w_e) * NQ + sq          # [E]
    cnt = np.bincount(gkey, minlength=8 * NW * NQ).reshape(8, NW, NQ)
    cap = cnt.max(axis=0)                        # [NW, NQ]

    # slot layout: sb -> quadrant -> window.  SWDGE gather ucode dies above
    # 1024 indices per call (empirical), so every call is <= 1024.
    MAXIDX = 1024
    wq_start = np.zeros((NW, NQ), np.int64)
    hcalls = []                                  # (sb, q, slot0, n_idx)
    sb_chunks = []                               # (k0, k1) per sb
    nslot = 0
    for s in range(NSB):
        k0 = nslot // 128
        for q in range(NQ):
            seg0 = nslot
            for w in range(s * NWSB, (s + 1) * NWSB):
                wq_start[w, q] = nslot
                nslot += int(cap[w, q])
            nslot = _ceil128(nslot)
            for off in range(seg0, nslot, MAXIDX):
                hcalls.append((s, q, off, min(MAXIDX, nslot - off)))
        sb_chunks.append((k0, nslot // 128))
    NSLOT = nslot
    NCHUNK = NSLOT // 128

    # dst-indexed gather calls (a_dst expansion), whole slot range
    adcalls = [(off, min(MAXIDX, NSLOT - off))
               for off in range(0, NSLOT, MAXIDX)]

    # per-window chunk columns
    colmap = np.full((NW, NCHUNK), -1, np.int64)
    wcols = []                                   # per w: (colbase, [chunks])
    ncol = 0
    for w in range(NW):
        cols = []
        for q in range(NQ):
            a = int(wq_start[w, q])
            b = a + int(cap[w, q])
            for k in range(a // 128, (b + 127) // 128):
                cols.append(k)
                colmap[w, k] = ncol
                ncol += 1
        wcols.append(cols)
    NCOL = ncol
    MAXCPW = max(len(c) for c in wcols)

    plan = {
        "NSLOT": NSLOT, "NCHUNK": NCHUNK, "NCOL": NCOL, "MAXCPW": MAXCPW,
        "hcalls": hcalls, "adcalls": adcalls, "sb_chunks": sb_chunks,
        "wcols": wcols,
        "skip_b1": bool(np.all(np.asarray(b1) == 0)),
        "skip_b2": bool(np.all(np.asarray(b2) == 0)),
    }

    # group-id in slot order: (sb, q, w_in_sb)
    flatg = (sb_e * NQ + sq) * NWSB + (w_e - sb_e * NWSB)
    gstart_flat = np.zeros(NSB * NQ * NWSB, np.int64)
    for s in range(NSB):
        for q in range(NQ):
            for wi in range(NWSB):
                gstart_flat[(s * NQ + q) * NWSB + wi] = wq_start[s * NWSB + wi, q]

    per_core = []
    hidx_all, adidx_all, drel_all = [], [], []
    for c in range(8):
        m = core == c
        fg = flatg[m]
        drc = dr_e[m]
        order = np.lexsort((drc, fg))
        fgs = fg[order]
        cntc = np.bincount(fgs, minlength=NSB * NQ * NWSB)
        starts = np.zeros_like(cntc)
        starts[1:] = np.cumsum(cntc)[:-1]
        rank = np.arange(len(fgs)) - starts[fgs]
        slot = gstart_flat[fgs] + rank

        hvec = np.zeros(NSLOT, np.int16)
        avec = np.zeros(NSLOT, np.int16)
        hvec[slot] = sidx[m][order]
        avec[slot] = adidx[m][order]

        drel = np.full((128, NCOL), 128.0, np.float32)
        k_s = slot >> 7
        p_s = slot & 127
        we_s = w_e[m][order]
        col_s = colmap[we_s, k_s]
        assert (col_s >= 0).all()
        drel[p_s, col_s] = drc[order].astype(np.float32)

        def wrap16(v):
            o = np.zeros((128, NSLOT // 16), np.int16)
            sl = np.arange(NSLOT)
            o[sl % 16, sl // 16] = v
            for r in range(1, 8):
                o[16 * r:16 * (r + 1)] = o[:16]
            return o

        hidx_all.append(wrap16(hvec))
        adidx_all.append(wrap16(avec))
        drel_all.append(drel.astype(ml_dtypes.bfloat16))

    # weights
    W1 = np.asarray(W1, np.float32)
    v_s1 = np.einsum("chk,hk->ch", W1.reshape(NF, HEADS, NHID),
                     np.asarray(a1_src, np.float32))
    v_d1 = np.einsum("chk,hk->ch", W1.reshape(NF, HEADS, NHID),
                     np.asarray(a1_dst, np.float32))
    W1e = np.concatenate([W1, v_s1, v_d1], axis=1).reshape(2, 128, 80)
    W1e = W1e.astype(np.float16)

    W2 = np.asarray(W2, np.float32)
    v_s2 = W2 @ np.asarray(a2_src, np.float32)[0]
    v_d2 = W2 @ np.asarray(a2_dst, np.float32)[0]
    W2e = np.concatenate([W2, v_s2[:, None], v_d2[:, None]],
                         axis=1).astype(np.float16)   # [64, 42]

    xp = np.zeros((NTOT, NF), np.float32)
    xp[:N] = np.asarray(x, np.float32)

    for c in range(8):
        xloc = np.ascontiguousarray(xp[c * NLOC:(c + 1) * NLOC].T)
        per_core.append({
            "xTloc": xloc.astype(np.float16).reshape(2, 128, NLOC),
            "W1e": W1e,
            "W2e": W2e,
            "b1": np.asarray(b1, np.float32)[None, :],
            "b2": np.asarray(b2, np.float32)[None, :],
            "hidx": hidx_all[c],
            "adidx": adidx_all[c],
            "drel": drel_all[c],
        })
    return per_core, plan


def _gather_small(g, out_ap, in_ap, idxs_ap, num_idxs, elem_size, elem_step,
                  queue_num=0):
    """dma_gather with payload < 256B; only the 256B row-stride rule is real
    for the non-transpose path."""
    import concourse.mybir as mybir
    stride_bytes = elem_step * mybir.dt.size(in_ap.dtype)
    assert stride_bytes % 256 == 0
    _in_ap = g.lower_ap_dma(in_ap, for_custom_bir_dma=True)
    _idxs_ap = g.lower_ap(idxs_ap)
    _out_ap = g.lower_ap(out_ap)
    return g.add_instruction(mybir.InstDMAGatherAnt(
        name=g.bass.get_next_instruction_name(),
        ins=[*_in_ap, _idxs_ap, g.lower_val_access(g.to_reg(num_idxs))],
        outs=[_out_ap],
        transpose=False,
        num_idxs=num_idxs,
        elem_size=elem_size,
        stride_bytes_256=stride_bytes // 256,
        gen_mode=0,
        single_packet=True,
        queue_num=queue_num,
        sbuf_tokens_per_rank=0,
        sbuf_free_dim_per_rank=0,
        sbuf_free_dim_pad_per_rank=0,
        sbuf_byte_offset=0,
    ))


def _build_nc(plan):
    import concourse.bass as bass
    import concourse.bacc as bacc
    import concourse.mybir as mybir
    import concourse.tile as tile
    from concourse.library_config import mlp
    from concourse.masks import make_identity

    f32, f16, bf16, i16 = (mybir.dt.float32, mybir.dt.float16,
                           mybir.dt.bfloat16, mybir.dt.int16)
    AF = mybir.ActivationFunctionType
    OP = mybir.AluOpType

    NSLOT = plan["NSLOT"]
    NCOL = plan["NCOL"]
    MAXCPW = plan["MAXCPW"]
    sb_chunks = plan["sb_chunks"]
    wcols = plan["wcols"]
    CPSB_MAX = max(k1 - k0 for k0, k1 in sb_chunks)
    colbase = [0] * NW
    for w in range(1, NW):
        colbase[w] = colbase[w - 1] + len(wcols[w - 1])

    nc = bacc.Bacc("TRN2", target_bir_lowering=False, debug=False,
                   num_devices=8, num_swdge_queues=4)

    xTloc = nc.dram_tensor("xTloc", [2, 128, NLOC], f16, kind="ExternalInput")
    W1e = nc.dram_tensor("W1e", [2, 128, 80], f16, kind="ExternalInput")
    W2e = nc.dram_tensor("W2e", [64, 42], f16, kind="ExternalInput")
    b1 = nc.dram_tensor("b1", [1, 64], f32, kind="ExternalInput")
    b2 = nc.dram_tensor("b2", [1, 40], f32, kind="ExternalInput")
    hidx_d = nc.dram_tensor("hidx", [128, NSLOT // 16], i16,
                            kind="ExternalInput")
    adidx_d = nc.dram_tensor("adidx", [128, NSLOT // 16], i16,
                             kind="ExternalInput")
    drel_d = nc.dram_tensor("drel", [128, NCOL], bf16, kind="ExternalInput")
    out = nc.dram_tensor("out", [NLOC, 40], f32, kind="ExternalOutput")

    agi1 = nc.dram_tensor("agi1", [NLOC, 128], f16)    # local [h1|as1|ad1|pad]
    tab1 = nc.dram_tensor("tab1", [NTOT, 128], f16, addr_space="Shared")
    agi2 = nc.dram_tensor("agi2", [NLOC, 128], f16)    # [h2|as2|ad2|pad]
    ago = nc.dram_tensor("ago", [NTOT, 128], f16, addr_space="Shared")

    def BC(ap, dims):
        return bass.AP(ap.tensor, ap.offset, dims)

    def dram_rows(t, offset_rows, dims):
        """AP into DRAM tensor t (row-major, 128 f16 cols) at row offset."""
        return bass.AP(t, offset_rows * 128, dims)

    with tile.TileContext(nc) as tc:
        with tc.tile_pool(name="const", bufs=1) as pc:
            nc.gpsimd.load_library(mlp)

            drel_sb = pc.tile([128, NCOL], bf16)
            nc.sync.dma_start(drel_sb[:], drel_d[:])
            w1_sb = pc.tile([128, 2, 80], f16)
            nc.sync.dma_start(w1_sb[:], W1e[:].rearrange("k p n -> p k n"))
            w2_sb = pc.tile([64, 42], f16)
            nc.sync.dma_start(w2_sb[:], W2e[:])

            ii = pc.tile([128, 128, MAXCPW], i16)
            nc.gpsimd.iota(ii[:], pattern=[[1, 128], [0, MAXCPW]], base=0,
                           channel_multiplier=0)
            iota_rep = pc.tile([128, 128, MAXCPW], bf16)
            nc.vector.tensor_copy(out=iota_rep[:], in_=ii[:])

            ident = pc.tile([128, 128], f16)
            make_identity(nc, ident[:])

            ones32 = pc.tile([1, 128], f32)
            nc.vector.memset(ones32[:], 1.0)

            b1r = pc.tile([128, 64], f32)
            b2r = pc.tile([128, 40], f32)
            with tc.tile_pool(name="pini", bufs=2, space="PSUM") as ppi:
                for row_d, width, dest in ((b1, 64, b1r), (b2, 40, b2r)):
                    t = pc.tile([1, width], f32, tag=f"rrow{width}")
                    nc.sync.dma_start(t[:], row_d[:])
                    ps = ppi.tile([128, width], f32, tag="rep")
                    nc.tensor.matmul(ps[:], lhsT=ones32[:], rhs=t[:],
                                     start=True, stop=True)
                    nc.vector.tensor_copy(out=dest[:], in_=ps[:])

            # a_dst per-slot tiles, gathered once and reused per layer
            at1_all = pc.tile([128, plan["NCHUNK"], 8], f16)
            at2_all = pc.tile([128, plan["NCHUNK"], 2], f16)

            # ---------- phase A (sharded): each core transforms only its own
            # node block -> agi1, AllGather -> tab1; the ad1-gather burst runs
            # during the collective (it reads the local agi1) ----------
            with (tc.tile_pool(name="pa", bufs=3) as pa,
                  tc.tile_pool(name="ppa", bufs=2, space="PSUM") as ppa):
                AB = 16                      # chunks per DMA batch
                for jj in range(0, 100, AB):
                    nb = min(AB, 100 - jj)
                    xt = pa.tile([128, 2, AB * 128], f16, tag="xt")
                    for k in range(2):
                        nc.sync.dma_start(
                            xt[:, k, 0:nb * 128],
                            xTloc[k, :, jj * 128:(jj + nb) * 128])
                    row = pa.tile([128, AB, 128], f16, tag="row")
                    for u in range(0, nb, 4):
                        ub = min(4, nb - u)
                        ps = ppa.tile([128, 4, 80], f32, tag="np1")
                        for j in range(ub):
                            for k in range(2):
                                nc.tensor.matmul(
                                    ps[:, j, :],
                                    lhsT=xt[:, k,
                                            (u + j) * 128:(u + j + 1) * 128],
                                    rhs=w1_sb[:, k, :], start=(k == 0),
                                    stop=(k == 1))
                        if (u // 4) % 2:
                            nc.vector.tensor_copy(out=row[:, u:u + ub, 0:80],
                                                  in_=ps[:, 0:ub, :])
                        else:
                            nc.scalar.copy(out=row[:, u:u + ub, 0:80],
                                           in_=ps[:, 0:ub, :])
                    nc.sync.dma_start(
                        dram_rows(agi1, jj,
                                  [[NW * 128, 128], [128, nb], [1, 128]]),
                        row[:, 0:nb, :])

                nc.gpsimd.collective_compute(
                    "AllGather", OP.bypass, ins=[agi1[:]], outs=[tab1[:]],
                    replica_groups=[list(range(8))])

                # ad1-gather burst (reads local agi1; overlaps the AllGather)
                aix = pa.tile([128, NSLOT // 16], i16, tag="aix")
                nc.sync.dma_start(aix[:], adidx_d[:])
                for ci, (slot0, nids) in enumerate(plan["adcalls"]):
                    _gather_small(
                        nc.gpsimd,
                        at1_all[:, slot0 // 128:(slot0 + nids) // 128, :],
                        agi1[:, 72:80],
                        aix[:, slot0 // 16:(slot0 + nids) // 16],
                        nids, 8, 128, queue_num=ci % 4)

            # ---------- phase B: layer-1 edge pass ----------
            def edge_pass(layer):
                if layer == 1:
                    tab, ncols_h, as_col = tab1, 72, 64
                    nheads, msgw = 8, 72
                else:
                    tab, ncols_h, as_col = ago, 41, 40
                    nheads, msgw = 1, 41
                pool_name = f"pe{layer}"
                with (tc.tile_pool(name=pool_name, bufs=2) as pb,
                      tc.tile_pool(name=pool_name + "h", bufs=3) as ph,
                      tc.tile_pool(name=pool_name + "m", bufs=1) as pm,
                      tc.tile_pool(name=pool_name + "w", bufs=2) as pw,
                      tc.tile_pool(name=pool_name + "p", bufs=2,
                                   space="PSUM") as ppb):
                    qn = 0
                    for s in range(NSB):
                        k0, k1 = sb_chunks[s]
                        cps = k1 - k0
                        hix = pb.tile([128, CPSB_MAX * 8], i16, tag="hix")
                        nc.sync.dma_start(hix[:, 0:cps * 8],
                                          hidx_d[:, k0 * 8:k1 * 8])

                        ht = ph.tile([128, CPSB_MAX, ncols_h], f16, tag="ht")
                        for (ss, q, slot0, nids) in plan["hcalls"]:
                            if ss != s:
                                continue
                            c0 = slot0 // 128 - k0
                            _gather_small(
                                nc.gpsimd,
                                ht[:, c0:c0 + nids // 128, :],
                                tab[q * QS:(q + 1) * QS, 0:ncols_h],
                                hix[:, (slot0 - k0 * 128) // 16:
                                    (slot0 - k0 * 128 + nids) // 16],
                                nids, ncols_h, 128, queue_num=qn % 4)
                            qn += 1
                        if layer == 1:
                            at_s = at1_all[:, k0:k1, :]
                        else:
                            at_s = at2_all[:, k0:k1, 1:2]

                        e = pm.tile([128, CPSB_MAX, nheads], f32, tag="e")
                        lr = e
                        nc.vector.tensor_tensor(
                            out=e[:, 0:cps, :],
                            in0=ht[:, 0:cps, as_col:as_col + nheads],
                            in1=at_s, op=OP.add)
                        nc.vector.scalar_tensor_tensor(
                            out=lr[:, 0:cps, :], in0=e[:, 0:cps, :],
                            scalar=0.2, in1=e[:, 0:cps, :],
                            op0=OP.mult, op1=OP.max)

                        msg = pm.tile([128, CPSB_MAX, msgw], bf16, tag="msg")
                        # w into msg's trailing cols (compact exp)
                        nc.scalar.activation(
                            out=msg[:, 0:cps, as_col:as_col + nheads],
                            in_=lr[:, 0:cps, :], func=AF.Exp)
                        if layer == 1:
                            # expanded weights for a clean 2x-mode mult
                            half = (CPSB_MAX + 1) // 2
                            wgx = pm.tile([128, half, 8, 8], bf16, tag="wgx")
                            for h0 in (0, half):
                                hn = min(half, cps - h0)
                                if hn <= 0:
                                    continue
                                lrs = lr[:, h0:h0 + hn, :]
                                nc.scalar.activation(
                                    out=wgx[:, 0:hn, :, :],
                                    in_=BC(lrs, [lrs.ap[0], lrs.ap[1],
                                                 lrs.ap[2], [0, 8]]),
                                    func=AF.Exp)
                                m_ = msg[:, h0:h0 + hn, 0:64]
                                h_ = ht[:, h0:h0 + hn, 0:64]
                                nc.vector.tensor_tensor(
                                    out=BC(m_, [m_.ap[0], m_.ap[1],
                                                [8, 8], [1, 8]]),
                                    in0=BC(h_, [h_.ap[0], h_.ap[1],
                                                [8, 8], [1, 8]]),
                                    in1=wgx[:, 0:hn, :, :], op=OP.mult)
                        else:
                            wgx2 = pw.tile([128, CPSB_MAX, 40], bf16,
                                           tag="wgx2")
                            lrs = lr[:, 0:cps, :]
                            nc.scalar.activation(
                                out=wgx2[:, 0:cps, :],
                                in_=BC(lrs, [lrs.ap[0], lrs.ap[1], [0, 40]]),
                                func=AF.Exp)
                            nc.vector.tensor_tensor(
                                out=msg[:, 0:cps, 0:40],
                                in0=ht[:, 0:cps, 0:40],
                                in1=wgx2[:, 0:cps, :], op=OP.mult)

                        # windows: one-hot + aggregation matmuls, PSUM
                        # evicted into a per-sb batch tile
                        hsb = pm.tile([128, NWSB, msgw], f32, tag="hsb")
                        for wi in range(NWSB):
                            w = s * NWSB + wi
                            cols = wcols[w]
                            cpw = len(cols)
                            c0 = colbase[w]
                            ohT = pw.tile([128, 128, MAXCPW], bf16, tag="ohT")
                            dr = drel_sb[:, c0:c0 + cpw]
                            nc.vector.tensor_tensor(
                                out=ohT[:, :, 0:cpw],
                                in0=BC(dr, [dr.ap[0], [0, 128], dr.ap[1]]),
                                in1=iota_rep[:, :, 0:cpw], op=OP.is_equal)
                            ps = ppb.tile([128, msgw], f32, tag="agg")
                            for i, k in enumerate(cols):
                                nc.tensor.matmul(
                                    ps[:], lhsT=ohT[:, :, i],
                                    rhs=msg[:, k - k0, :],
                                    start=(i == 0), stop=(i == cpw - 1))
                            nc.scalar.copy(out=hsb[:, wi, :], in_=ps[:])

                        # per-sb batched softmax-normalize (+ elu/r2 for L1)
                        if layer == 1:
                            den = pw.tile([128, NWSB, 8], f32, tag="den")
                            nc.scalar.activation(out=den[:],
                                                 in_=hsb[:, :, 64:72],
                                                 func=AF.Copy, bias=ACC_EPS)
                            rec = pw.tile([128, NWSB, 8], f32, tag="rec")
                            nc.vector.reciprocal(
                                rec[:].rearrange("p a b -> p (a b)"),
                                den[:].rearrange("p a b -> p (a b)"))
                            o1 = pw.tile([128, NWSB, 64], f32, tag="o1")
                            nu = hsb[:, :, 0:64]
                            r_ = rec[:]
                            nc.vector.tensor_tensor(
                                out=BC(o1[:], [o1[:].ap[0], [64, NWSB],
                                               [8, 8], [1, 8]]),
                                in0=BC(nu, [nu.ap[0], [72, NWSB],
                                            [8, 8], [1, 8]]),
                                in1=BC(r_, [r_.ap[0], [8, NWSB],
                                            [1, 8], [0, 8]]),
                                op=OP.mult)
                            o1v = o1[:].rearrange("p a b -> p (a b)")
                            if not plan["skip_b1"]:
                                b1w = b1r[:]
                                nc.vector.tensor_tensor(
                                    out=o1v,
                                    in0=o1v,
                                    in1=BC(b1w, [b1w.ap[0], [0, NWSB],
                                                 [1, 64]]),
                                    op=OP.add)
                            # elu = relu(x) + exp(-relu(-x)) - 1
                            rneg = pw.tile([128, NWSB, 64], f32, tag="rneg")
                            nc.scalar.activation(
                                out=rneg[:].rearrange("p a b -> p (a b)"),
                                in_=o1v, func=AF.Relu, scale=-1.0)
                            expn = rneg
                            nc.scalar.activation(
                                out=expn[:].rearrange("p a b -> p (a b)"),
                                in_=rneg[:].rearrange("p a b -> p (a b)"),
                                func=AF.Exp, scale=-1.0)
                            pos = pw.tile([128, NWSB, 64], f32, tag="pos")
                            nc.scalar.activation(
                                out=pos[:].rearrange("p a b -> p (a b)"),
                                in_=o1v, func=AF.Relu)
                            hl16 = pw.tile([128, NWSB, 64], f16, tag="hl16")
                            nc.vector.scalar_tensor_tensor(
                                out=hl16[:].rearrange("p a b -> p (a b)"),
                                in0=expn[:].rearrange("p a b -> p (a b)"),
                                scalar=-1.0,
                                in1=pos[:].rearrange("p a b -> p (a b)"),
                                op0=OP.add, op1=OP.add)
                            r2s = pw.tile([128, NWSB, 42], f16, tag="r2s")
                            for wi in range(NWSB):
                                pst = ppb.tile([64, 128], f16, tag="tr")
                                nc.tensor.transpose(out=pst[:],
                                                    in_=hl16[:, wi, :],
                                                    identity=ident[:])
                                hlT = pw.tile([64, 128], f16, tag="hlT")
                                nc.scalar.copy(out=hlT[:], in_=pst[:])
                                r2p = ppb.tile([128, 42], f32, tag="r2p")
                                nc.tensor.matmul(r2p[:], lhsT=hlT[:],
                                                 rhs=w2_sb[:], start=True,
                                                 stop=True)
                                nc.scalar.copy(out=r2s[:, wi, :], in_=r2p[:])
                            nc.sync.dma_start(
                                bass.AP(agi2, (s * NWSB) * 128,
                                        [[NW * 128, 128], [128, NWSB],
                                         [1, 42]]),
                                r2s[:])
                        else:
                            den = pw.tile([128, NWSB, 1], f32, tag="den2")
                            nc.scalar.activation(out=den[:],
                                                 in_=hsb[:, :, 40:41],
                                                 func=AF.Copy, bias=ACC_EPS)
                            rec = pw.tile([128, NWSB, 1], f32, tag="rec2")
                            nc.vector.reciprocal(
                                rec[:].rearrange("p a b -> p (a b)"),
                                den[:].rearrange("p a b -> p (a b)"))
                            o2 = pw.tile([128, NWSB, 40], f32, tag="o2")
                            nu = hsb[:, :, 0:40]
                            r_ = rec[:]
                            nc.vector.tensor_tensor(
                                out=o2[:],
                                in0=BC(nu, [nu.ap[0], [41, NWSB], [1, 40]]),
                                in1=BC(r_, [r_.ap[0], [1, NWSB], [0, 40]]),
                                op=OP.mult)
                            o2v = o2[:].rearrange("p a b -> p (a b)")
                            if not plan["skip_b2"]:
                                b2w = b2r[:]
                                nc.vector.tensor_tensor(
                                    out=o2v, in0=o2v,
                                    in1=BC(b2w, [b2w.ap[0], [0, NWSB],
                                                 [1, 40]]),
                                    op=OP.add)
                            nc.sync.dma_start(
                                bass.AP(out, (s * NWSB) * 128 * 40,
                                        [[40, 128], [128 * 40, NWSB],
                                         [1, 40]]),
                                o2[:])

            edge_pass(1)

            # ---------- AllGather first (Pool dispatches it, then keeps
            # generating ad2-gather descriptors while it runs) ----------
            nc.gpsimd.collective_compute(
                "AllGather", OP.bypass, ins=[agi2[:]], outs=[ago[:]],
                replica_groups=[list(range(8))])

            # ---------- ad2-gather burst (overlaps the AllGather) ----------
            with tc.tile_pool(name="pad2", bufs=1) as pd2:
                aix2 = pd2.tile([128, NSLOT // 16], i16)
                nc.sync.dma_start(aix2[:], adidx_d[:])
                for ci, (slot0, nids) in enumerate(plan["adcalls"]):
                    _gather_small(
                        nc.gpsimd,
                        at2_all[:, slot0 // 128:(slot0 + nids) // 128, :],
                        agi2[:, 40:42],
                        aix2[:, slot0 // 16:(slot0 + nids) // 16],
                        nids, 2, 128, queue_num=ci % 4)

            edge_pass(2)

    nc.finalize()
    return nc


def kernel(**inputs):
    per_core, plan = _host_prep(**inputs)
    if "nc" not in _CACHE:
        _CACHE["nc"] = _build_nc(plan)
    nc = _CACHE["nc"]
    from concourse.bass_utils import run_bass_kernel_spmd
    res = run_bass_kernel_spmd(nc, per_core, list(range(8)))
    full = np.concatenate([res.results[c]["out"] for c in range(8)], axis=0)
    return np.ascontiguousarray(full[:N]).astype(np.float32)


# revision 38
# speedup vs baseline: 2.3253x; 1.0015x over previous
"""GAT (2-layer PyG GATConv, eval) on 8 Trainium2 NeuronCores.

Sharding: nodes range-partitioned (NLOC=12800/core); core c owns edges whose
dst is in its range. Both layers' node tables are computed SHARDED (each core
transforms only its own 12800-node block) and replicated by one AllGather per
layer; each AllGather is fully overlapped by the a_dst gather burst for the
next edge pass, which reads only the local block.

Slot layout per core: superblock (10 windows) -> quadrant -> window, with
per-(window,quadrant) STATIC capacities = max edge count over the 8 cores
(SPMD: one module runs on all cores; only tensor contents differ) — ~10%
slot padding vs 28% for fixed-size groups. Every dma_gather call carries at
most 1024 indices (the SWDGE ucode's ring limit on real hardware; larger
calls hard-crash the device), rotated across the 4 SWDGE queues.

Per layer, per edge slot: a 144B/82B payload gather pulls [h|a_src] rows
(256B-stride tables, int16 idx into 25600-row quadrants); a 16B/4B gather
pulls a_dst. Softmax is the shift-invariant no-max form
(w = exp(leakyrelu(as+ad)), |e| < ~25 so fp32 exp is safe). Segment reduction
is a PE matmul whose stationary matrix is a transposed one-hot built by DVE
is_equal in the 2x-mode layout (both operands packed 2-byte, last-dim stride
1); chunk slices address it with a strided free dim, boundary chunks carry a
masked drel column per touching window. Messages msg = h*w use an
Act-expanded weight tile so the DVE mult also runs in 2x mode. Softmax
normalize + ELU + the r2 = hlT @ [W2|a2_src|a2_dst] projection are batched
per superblock; tables are written node-permuted (row = p*100 + j within
each core block) so table writes coalesce to one descriptor per partition.
"""
import numpy as np
import ml_dtypes

N = 100000
E = 1600000
NF = 256
HEADS, NHID = 8, 8
NH = HEADS * NHID          # 64
NCLASS = 40
NLOC = 12800               # nodes per core
NW = 100                   # 128-dst windows per core
NQ = 4                     # src table quadrants
QS = 25600                 # rows per quadrant
NWSB = 10                  # windows per superblock
NSB = NW // NWSB           # 10 superblocks
NTOT = 102400
ACC_EPS = 1e-16

_CACHE = {}


def _ceil128(x):
    return (x + 127) & ~127


def _host_prep(x, edge_index, W1, a1_src, a1_dst, b1, W2, a2_src, a2_dst, b2):
    src = np.asarray(edge_index[0], dtype=np.int64)
    dst = np.asarray(edge_index[1], dtype=np.int64)

    # table-row permutation: node n -> row  c*NLOC + (l%128)*NW + l//128
    def rowperm(n):
        c = n // NLOC
        l = n - c * NLOC
        return c * NLOC + (l % 128) * NW + l // 128

    srow = rowperm(src)
    sq = srow // QS
    sidx = (srow - sq * QS).astype(np.int16)

    core = dst // NLOC
    dloc = dst - core * NLOC
    w_e = (dloc >> 7).astype(np.int64)
    dr_e = (dloc & 127).astype(np.int64)
    adidx = (dr_e * NW + w_e).astype(np.int16)
    sb_e = w_e // NWSB

    # static capacities: max over cores per (window, quadrant)
    gkey = (core * NW + w_e) * NQ + sq          # [E]
    cnt = np.bincount(gkey, minlength=8 * NW * NQ).reshape(8, NW, NQ)
    cap = cnt.max(axis=0)                        # [NW, NQ]

    # slot layout: sb -> quadrant -> window.  SWDGE gather ucode dies above
    # 1024 indices per call (empirical), so every call is <= 1024.
    MAXIDX = 1024
    wq_start = np.zeros((NW, NQ), np.int64)
    hcalls = []                                  # (sb, q, slot0, n_idx)
    sb_chunks = []                               # (k0, k1) per sb
    nslot = 0
    for s in range(NSB):
        k0 = nslot // 128
        for q in range(NQ):
            seg0 = nslot
            for w in range(s * NWSB, (s + 1) * NWSB):
                wq_start[w, q] = nslot
                nslot += int(cap[w, q])
            nslot = _ceil128(nslot)
            for off in range(seg0, nslot, MAXIDX):
                hcalls.append((s, q, off, min(MAXIDX, nslot - off)))
        sb_chunks.append((k0, nslot // 128))
    NSLOT = nslot
    NCHUNK = NSLOT // 128

    # dst-indexed gather calls (a_dst expansion), whole slot range
    adcalls = [(off, min(MAXIDX, NSLOT - off))
               for off in range(0, NSLOT, MAXIDX)]

    # per-window chunk columns
    colmap = np.full((NW, NCHUNK), -1, np.int64)
    wcols = []                                   # per w: (colbase, [chunks])
    ncol = 0
    for w in range(NW):
        cols = []
        for q in range(NQ):
            a = int(wq_start[w, q])
            b = a + int(cap[w, q])
            for k in range(a // 128, (b + 127) // 128):
                cols.append(k)
                colmap[w, k] = ncol
                ncol += 1
        wcols.append(cols)
    NCOL = ncol
    MAXCPW = max(len(c) for c in wcols)

    plan = {
        "NSLOT": NSLOT, "NCHUNK": NCHUNK, "NCOL": NCOL, "MAXCPW": MAXCPW,
        "hcalls": hcalls, "adcalls": adcalls, "sb_chunks": sb_chunks,
        "wcols": wcols,
        "skip_b1": bool(np.all(np.asarray(b1) == 0)),
        "skip_b2": bool(np.all(np.asarray(b2) == 0)),
    }

    # group-id in slot order: (sb, q, w_in_sb)
    flatg = (sb_e * NQ + sq) * NWSB + (w_e - sb_e * NWSB)
    gstart_flat = np.zeros(NSB * NQ * NWSB, np.int64)
    for s in range(NSB):
        for q in range(NQ):
            for wi in range(NWSB):
                gstart_flat[(s * NQ + q) * NWSB + wi] = wq_start[s * NWSB + wi, q]

    per_core = []
    hidx_all, adidx_all, drel_all = [], [], []
    for c in range(8):
        m = core == c
        fg = flatg[m]
        drc = dr_e[m]
        order = np.lexsort((drc, fg))
        fgs = fg[order]
        cntc = np.bincount(fgs, minlength=NSB * NQ * NWSB)
        starts = np.zeros_like(cntc)
        starts[1:] = np.cumsum(cntc)[:-1]
        rank = np.arange(len(fgs)) - starts[fgs]
        slot = gstart_flat[fgs] + rank

        hvec = np.zeros(NSLOT, np.int16)
        avec = np.zeros(NSLOT, np.int16)
        hvec[slot] = sidx[m][order]
        avec[slot] = adidx[m][order]

        drel = np.full((128, NCOL), 128.0, np.float32)
        k_s = slot >> 7
        p_s = slot & 127
        we_s = w_e[m][order]
        col_s = colmap[we_s, k_s]
        assert (col_s >= 0).all()
        drel[p_s, col_s] = drc[order].astype(np.float32)

        def wrap16(v):
            o = np.zeros((128, NSLOT // 16), np.int16)
            sl = np.arange(NSLOT)
            o[sl % 16, sl // 16] = v
            for r in range(1, 8):
                o[16 * r:16 * (r + 1)] = o[:16]
            return o

        hidx_all.append(wrap16(hvec))
        adidx_all.append(wrap16(avec))
        drel_all.append(drel.astype(ml_dtypes.bfloat16))

    # weights
    W1 = np.asarray(W1, np.float32)
    v_s1 = np.einsum("chk,hk->ch", W1.reshape(NF, HEADS, NHID),
                     np.asarray(a1_src, np.float32))
    v_d1 = np.einsum("chk,hk->ch", W1.reshape(NF, HEADS, NHID),
                     np.asarray(a1_dst, np.float32))
    W1e = np.concatenate([W1, v_s1, v_d1], axis=1).reshape(2, 128, 80)
    W1e = W1e.astype(np.float16)

    W2 = np.asarray(W2, np.float32)
    v_s2 = W2 @ np.asarray(a2_src, np.float32)[0]
    v_d2 = W2 @ np.asarray(a2_dst, np.float32)[0]
    W2e = np.concatenate([W2, v_s2[:, None], v_d2[:, None]],
                         axis=1).astype(np.float16)   # [64, 42]

    xp = np.zeros((NTOT, NF), np.float32)
    xp[:N] = np.asarray(x, np.float32)

    for c in range(8):
        xloc = np.ascontiguousarray(xp[c * NLOC:(c + 1) * NLOC].T)
        per_core.append({
            "xTloc": xloc.astype(np.float16).reshape(2, 128, NLOC),
            "W1e": W1e,
            "W2e": W2e,
            "b1": np.asarray(b1, np.float32)[None, :],
            "b2": np.asarray(b2, np.float32)[None, :],
            "hidx": hidx_all[c],
            "adidx": adidx_all[c],
            "drel": drel_all[c],
        })
    return per_core, plan


def _gather_small(g, out_ap, in_ap, idxs_ap, num_idxs, elem_size, elem_step,
                  queue_num=0):
    """dma_gather with payload < 256B; only the 256B row-stride rule is real
    for the non-transpose path."""
    import concourse.mybir as mybir
    stride_bytes = elem_step * mybir.dt.size(in_ap.dtype)
    assert stride_bytes % 256 == 0
    _in_ap = g.lower_ap_dma(in_ap, for_custom_bir_dma=True)
    _idxs_ap = g.lower_ap(idxs_ap)
    _out_ap = g.lower_ap(out_ap)
    return g.add_instruction(mybir.InstDMAGatherAnt(
        name=g.bass.get_next_instruction_name(),
        ins=[*_in_ap, _idxs_ap, g.lower_val_access(g.to_reg(num_idxs))],
        outs=[_out_ap],
        transpose=False,
        num_idxs=num_idxs,
        elem_size=elem_size,
        stride_bytes_256=stride_bytes // 256,
        gen_mode=0,
        single_packet=True,
        queue_num=queue_num,
        sbuf_tokens_per_rank=0,
        sbuf_free_dim_per_rank=0,
        sbuf_free_dim_pad_per_rank=0,
        sbuf_byte_offset=0,
    ))


def _build_nc(plan):
    import concourse.bass as bass
    import concourse.bacc as bacc
    import concourse.mybir as mybir
    import concourse.tile as tile
    from concourse.library_config import mlp
    from concourse.masks import make_identity

    f32, f16, bf16, i16 = (mybir.dt.float32, mybir.dt.float16,
                           mybir.dt.bfloat16, mybir.dt.int16)
    AF = mybir.ActivationFunctionType
    OP = mybir.AluOpType

    NSLOT = plan["NSLOT"]
    NCOL = plan["NCOL"]
    MAXCPW = plan["MAXCPW"]
    sb_chunks = plan["sb_chunks"]
    wcols = plan["wcols"]
    CPSB_MAX = max(k1 - k0 for k0, k1 in sb_chunks)
    colbase = [0] * NW
    for w in range(1, NW):
        colbase[w] = colbase[w - 1] + len(wcols[w - 1])

    nc = bacc.Bacc("TRN2", target_bir_lowering=False, debug=False,
                   num_devices=8, num_swdge_queues=4)

    xTloc = nc.dram_tensor("xTloc", [2, 128, NLOC], f16, kind="ExternalInput")
    W1e = nc.dram_tensor("W1e", [2, 128, 80], f16, kind="ExternalInput")
    W2e = nc.dram_tensor("W2e", [64, 42], f16, kind="ExternalInput")
    b1 = nc.dram_tensor("b1", [1, 64], f32, kind="ExternalInput")
    b2 = nc.dram_tensor("b2", [1, 40], f32, kind="ExternalInput")
    hidx_d = nc.dram_tensor("hidx", [128, NSLOT // 16], i16,
                            kind="ExternalInput")
    adidx_d = nc.dram_tensor("adidx", [128, NSLOT // 16], i16,
                             kind="ExternalInput")
    drel_d = nc.dram_tensor("drel", [128, NCOL], bf16, kind="ExternalInput")
    out = nc.dram_tensor("out", [NLOC, 40], f32, kind="ExternalOutput")

    agi1 = nc.dram_tensor("agi1", [NLOC, 128], f16)    # local [h1|as1|ad1|pad]
    tab1 = nc.dram_tensor("tab1", [NTOT, 128], f16, addr_space="Shared")
    agi2 = nc.dram_tensor("agi2", [NLOC, 128], f16)    # [h2|as2|ad2|pad]
    ago = nc.dram_tensor("ago", [NTOT, 128], f16, addr_space="Shared")

    def BC(ap, dims):
        return bass.AP(ap.tensor, ap.offset, dims)

    def dram_rows(t, offset_rows, dims):
        """AP into DRAM tensor t (row-major, 128 f16 cols) at row offset."""
        return bass.AP(t, offset_rows * 128, dims)

    with tile.TileContext(nc) as tc:
        with tc.tile_pool(name="const", bufs=1) as pc:
            nc.gpsimd.load_library(mlp)

            drel_sb = pc.tile([128, NCOL], bf16)
            nc.sync.dma_start(drel_sb[:], drel_d[:])
            w1_sb = pc.tile([128, 2, 80], f16)
            nc.sync.dma_start(w1_sb[:], W1e[:].rearrange("k p n -> p k n"))
            w2_sb = pc.tile([64, 42], f16)
            nc.sync.dma_start(w2_sb[:], W2e[:])

            ii = pc.tile([128, 128, MAXCPW], i16)
            nc.gpsimd.iota(ii[:], pattern=[[1, 128], [0, MAXCPW]], base=0,
                           channel_multiplier=0)
            iota_rep = pc.tile([128, 128, MAXCPW], bf16)
            nc.vector.tensor_copy(out=iota_rep[:], in_=ii[:])

            ident = pc.tile([128, 128], f16)
            make_identity(nc, ident[:])

            ones32 = pc.tile([1, 128], f32)
            nc.vector.memset(ones32[:], 1.0)

            b1r = pc.tile([128, 64], f32)
            b2r = pc.tile([128, 40], f32)
            with tc.tile_pool(name="pini", bufs=2, space="PSUM") as ppi:
                for row_d, width, dest in ((b1, 64, b1r), (b2, 40, b2r)):
                    t = pc.tile([1, width], f32, tag=f"rrow{width}")
                    nc.sync.dma_start(t[:], row_d[:])
                    ps = ppi.tile([128, width], f32, tag="rep")
                    nc.tensor.matmul(ps[:], lhsT=ones32[:], rhs=t[:],
                                     start=True, stop=True)
                    nc.vector.tensor_copy(out=dest[:], in_=ps[:])

            # a_dst per-slot tiles, gathered once and reused per layer
            at1_all = pc.tile([128, plan["NCHUNK"], 8], f16)
            at2_all = pc.tile([128, plan["NCHUNK"], 2], f16)

            # ---------- phase A (sharded): each core transforms only its own
            # node block -> agi1, AllGather -> tab1; the ad1-gather burst runs
            # during the collective (it reads the local agi1) ----------
            with (tc.tile_pool(name="pa", bufs=3) as pa,
                  tc.tile_pool(name="ppa", bufs=2, space="PSUM") as ppa):
                AB = 16                      # chunks per DMA batch
                for jj in range(0, 100, AB):
                    nb = min(AB, 100 - jj)
                    xt = pa.tile([128, 2, AB * 128], f16, tag="xt")
                    for k in range(2):
                        nc.sync.dma_start(
                            xt[:, k, 0:nb * 128],
                            xTloc[k, :, jj * 128:(jj + nb) * 128])
                    row = pa.tile([128, AB, 128], f16, tag="row")
                    for u in range(0, nb, 4):
                        ub = min(4, nb - u)
                        ps = ppa.tile([128, 4, 80], f32, tag="np1")
                        for j in range(ub):
                            for k in range(2):
                                nc.tensor.matmul(
                                    ps[:, j, :],
                                    lhsT=xt[:, k,
                                            (u + j) * 128:(u + j + 1) * 128],
                                    rhs=w1_sb[:, k, :], start=(k == 0),
                                    stop=(k == 1))
                        if (u // 4) % 2:
                            nc.vector.tensor_copy(out=row[:, u:u + ub, 0:80],
                                                  in_=ps[:, 0:ub, :])
                        else:
                            nc.scalar.copy(out=row[:, u:u + ub, 0:80],
                                           in_=ps[:, 0:ub, :])
                    nc.sync.dma_start(
                        dram_rows(agi1, jj,
                                  [[NW * 128, 128], [128, nb], [1, 128]]),
                        row[:, 0:nb, :])

                nc.gpsimd.collective_compute(
                    "AllGather", OP.bypass, ins=[agi1[:]], outs=[tab1[:]],
                    replica_groups=[list(range(8))])

                # ad1-gather burst (reads local agi1; overlaps the AllGather)
                aix = pa.tile([128, NSLOT // 16], i16, tag="aix")
                nc.sync.dma_start(aix[:], adidx_d[:])
                for ci, (slot0, nids) in enumerate(plan["adcalls"]):
                    _gather_small(
                        nc.gpsimd,
                        at1_all[:, slot0 // 128:(slot0 + nids) // 128, :],
                        agi1[:, 72:80],
                        aix[:, slot0 // 16:(slot0 + nids) // 16],
                        nids, 8, 128, queue_num=ci % 4)

            # ---------- phase B: layer-1 edge pass ----------
            def edge_pass(layer):
                if layer == 1:
                    tab, ncols_h, as_col = tab1, 72, 64
                    nheads, msgw = 8, 72
                else:
                    tab, ncols_h, as_col = ago, 41, 40
                    nheads, msgw = 1, 41
                pool_name = f"pe{layer}"
                with (tc.tile_pool(name=pool_name, bufs=2) as pb,
                      tc.tile_pool(name=pool_name + "h", bufs=3) as ph,
                      tc.tile_pool(name=pool_name + "m", bufs=1) as pm,
                      tc.tile_pool(name=pool_name + "w",
                                   bufs=(2 if layer == 1 else 3)) as pw,
                      tc.tile_pool(name=pool_name + "p", bufs=2,
                                   space="PSUM") as ppb):
                    qn = 0
                    for s in range(NSB):
                        k0, k1 = sb_chunks[s]
                        cps = k1 - k0
                        hix = pb.tile([128, CPSB_MAX * 8], i16, tag="hix")
                        nc.sync.dma_start(hix[:, 0:cps * 8],
                                          hidx_d[:, k0 * 8:k1 * 8])

                        ht = ph.tile([128, CPSB_MAX, ncols_h], f16, tag="ht")
                        for (ss, q, slot0, nids) in plan["hcalls"]:
                            if ss != s:
                                continue
                            c0 = slot0 // 128 - k0
                            _gather_small(
                                nc.gpsimd,
                                ht[:, c0:c0 + nids // 128, :],
                                tab[q * QS:(q + 1) * QS, 0:ncols_h],
                                hix[:, (slot0 - k0 * 128) // 16:
                                    (slot0 - k0 * 128 + nids) // 16],
                                nids, ncols_h, 128, queue_num=qn % 4)
                            qn += 1
                        if layer == 1:
                            at_s = at1_all[:, k0:k1, :]
                        else:
                            at_s = at2_all[:, k0:k1, 1:2]

                        e = pm.tile([128, CPSB_MAX, nheads], f32, tag="e")
                        lr = e
                        nc.vector.tensor_tensor(
                            out=e[:, 0:cps, :],
                            in0=ht[:, 0:cps, as_col:as_col + nheads],
                            in1=at_s, op=OP.add)
                        nc.vector.scalar_tensor_tensor(
                            out=lr[:, 0:cps, :], in0=e[:, 0:cps, :],
                            scalar=0.2, in1=e[:, 0:cps, :],
                            op0=OP.mult, op1=OP.max)

                        msg = pm.tile([128, CPSB_MAX, msgw], bf16, tag="msg")
                        # w into msg's trailing cols (compact exp)
                        nc.scalar.activation(
                            out=msg[:, 0:cps, as_col:as_col + nheads],
                            in_=lr[:, 0:cps, :], func=AF.Exp)
                        if layer == 1:
                            # expanded weights for a clean 2x-mode mult
                            half = (CPSB_MAX + 1) // 2
                            wgx = pm.tile([128, half, 8, 8], bf16, tag="wgx")
                            for h0 in (0, half):
                                hn = min(half, cps - h0)
                                if hn <= 0:
                                    continue
                                lrs = lr[:, h0:h0 + hn, :]
                                nc.scalar.activation(
                                    out=wgx[:, 0:hn, :, :],
                                    in_=BC(lrs, [lrs.ap[0], lrs.ap[1],
                                                 lrs.ap[2], [0, 8]]),
                                    func=AF.Exp)
                                m_ = msg[:, h0:h0 + hn, 0:64]
                                h_ = ht[:, h0:h0 + hn, 0:64]
                                nc.vector.tensor_tensor(
                                    out=BC(m_, [m_.ap[0], m_.ap[1],
                                                [8, 8], [1, 8]]),
                                    in0=BC(h_, [h_.ap[0], h_.ap[1],
                                                [8, 8], [1, 8]]),
                                    in1=wgx[:, 0:hn, :, :], op=OP.mult)
                        else:
                            wgx2 = pw.tile([128, CPSB_MAX, 40], bf16,
                                           tag="wgx2")
                            lrs = lr[:, 0:cps, :]
                            nc.scalar.activation(
                                out=wgx2[:, 0:cps, :],
                                in_=BC(lrs, [lrs.ap[0], lrs.ap[1], [0, 40]]),
                                func=AF.Exp)
                            nc.vector.tensor_tensor(
                                out=msg[:, 0:cps, 0:40],
                                in0=ht[:, 0:cps, 0:40],
                                in1=wgx2[:, 0:cps, :], op=OP.mult)

                        # windows: one-hot + aggregation matmuls, PSUM
                        # evicted into a per-sb batch tile
                        hsb = pm.tile([128, NWSB, msgw], f32, tag="hsb")
                        for wi in range(NWSB):
                            w = s * NWSB + wi
                            cols = wcols[w]
                            cpw = len(cols)
                            c0 = colbase[w]
                            ohT = pw.tile([128, 128, MAXCPW], bf16, tag="ohT")
                            dr = drel_sb[:, c0:c0 + cpw]
                            nc.vector.tensor_tensor(
                                out=ohT[:, :, 0:cpw],
                                in0=BC(dr, [dr.ap[0], [0, 128], dr.ap[1]]),
                                in1=iota_rep[:, :, 0:cpw], op=OP.is_equal)
                            ps = ppb.tile([128, msgw], f32, tag="agg")
                            for i, k in enumerate(cols):
                                nc.tensor.matmul(
                                    ps[:], lhsT=ohT[:, :, i],
                                    rhs=msg[:, k - k0, :],
                                    start=(i == 0), stop=(i == cpw - 1))
                            nc.scalar.copy(out=hsb[:, wi, :], in_=ps[:])

                        # per-sb batched softmax-normalize (+ elu/r2 for L1)
                        if layer == 1:
                            den = pw.tile([128, NWSB, 8], f32, tag="den")
                            nc.scalar.activation(out=den[:],
                                                 in_=hsb[:, :, 64:72],
                                                 func=AF.Copy, bias=ACC_EPS)
                            rec = pw.tile([128, NWSB, 8], f32, tag="rec")
                            nc.vector.reciprocal(
                                rec[:].rearrange("p a b -> p (a b)"),
                                den[:].rearrange("p a b -> p (a b)"))
                            o1 = pw.tile([128, NWSB, 64], f32, tag="o1")
                            nu = hsb[:, :, 0:64]
                            r_ = rec[:]
                            nc.vector.tensor_tensor(
                                out=BC(o1[:], [o1[:].ap[0], [64, NWSB],
                                               [8, 8], [1, 8]]),
                                in0=BC(nu, [nu.ap[0], [72, NWSB],
                                            [8, 8], [1, 8]]),
                                in1=BC(r_, [r_.ap[0], [8, NWSB],
                                            [1, 8], [0, 8]]),
                                op=OP.mult)
                            o1v = o1[:].rearrange("p a b -> p (a b)")
                            if not plan["skip_b1"]:
                                b1w = b1r[:]
                                nc.vector.tensor_tensor(
                                    out=o1v,
                                    in0=o1v,
                                    in1=BC(b1w, [b1w.ap[0], [0, NWSB],
                                                 [1, 64]]),
                                    op=OP.add)
                            # elu = relu(x) + exp(-relu(-x)) - 1
                            rneg = pw.tile([128, NWSB, 64], f32, tag="rneg")
                            nc.scalar.activation(
                                out=rneg[:].rearrange("p a b -> p (a b)"),
                                in_=o1v, func=AF.Relu, scale=-1.0)
                            expn = rneg
                            nc.scalar.activation(
                                out=expn[:].rearrange("p a b -> p (a b)"),
                                in_=rneg[:].rearrange("p a b -> p (a b)"),
                                func=AF.Exp, scale=-1.0)
                            pos = pw.tile([128, NWSB, 64], f32, tag="pos")
                            nc.scalar.activation(
                                out=pos[:].rearrange("p a b -> p (a b)"),
                                in_=o1v, func=AF.Relu)
                            hl16 = pw.tile([128, NWSB, 64], f16, tag="hl16")
                            nc.vector.scalar_tensor_tensor(
                                out=hl16[:].rearrange("p a b -> p (a b)"),
                                in0=expn[:].rearrange("p a b -> p (a b)"),
                                scalar=-1.0,
                                in1=pos[:].rearrange("p a b -> p (a b)"),
                                op0=OP.add, op1=OP.add)
                            r2s = pw.tile([128, NWSB, 42], f16, tag="r2s")
                            for wi in range(NWSB):
                                pst = ppb.tile([64, 128], f16, tag="tr")
                                nc.tensor.transpose(out=pst[:],
                                                    in_=hl16[:, wi, :],
                                                    identity=ident[:])
                                hlT = pw.tile([64, 128], f16, tag="hlT")
                                nc.scalar.copy(out=hlT[:], in_=pst[:])
                                r2p = ppb.tile([128, 42], f32, tag="r2p")
                                nc.tensor.matmul(r2p[:], lhsT=hlT[:],
                                                 rhs=w2_sb[:], start=True,
                                                 stop=True)
                                nc.scalar.copy(out=r2s[:, wi, :], in_=r2p[:])
                            nc.sync.dma_start(
                                bass.AP(agi2, (s * NWSB) * 128,
                                        [[NW * 128, 128], [128, NWSB],
                                         [1, 42]]),
                                r2s[:])
                        else:
                            den = pw.tile([128, NWSB, 1], f32, tag="den2")
                            nc.scalar.activation(out=den[:],
                                                 in_=hsb[:, :, 40:41],
                                                 func=AF.Copy, bias=ACC_EPS)
                            rec = pw.tile([128, NWSB, 1], f32, tag="rec2")
                            nc.vector.reciprocal(
                                rec[:].rearrange("p a b -> p (a b)"),
                                den[:].rearrange("p a b -> p (a b)"))
                            o2 = pw.tile([128, NWSB, 40], f32, tag="o2")
                            nu = hsb[:, :, 0:40]
                            r_ = rec[:]
                            nc.vector.tensor_tensor(
                                out=o2[:],
                                in0=BC(nu, [nu.ap[0], [41, NWSB], [1, 40]]),
                                in1=BC(r_, [r_.ap[0], [1, NWSB], [0, 40]]),
                                op=OP.mult)
                            o2v = o2[:].rearrange("p a b -> p (a b)")
                            if not plan["skip_b2"]:
                                b2w = b2r[:]
                                nc.vector.tensor_tensor(
                                    out=o2v, in0=o2v,
                                    in1=BC(b2w, [b2w.ap[0], [0, NWSB],
                                                 [1, 40]]),
                                    op=OP.add)
                            nc.sync.dma_start(
                                bass.AP(out, (s * NWSB) * 128 * 40,
                                        [[40, 128], [128 * 40, NWSB],
                                         [1, 40]]),
                                o2[:])

            edge_pass(1)

            # ---------- AllGather first (Pool dispatches it, then keeps
            # generating ad2-gather descriptors while it runs) ----------
            nc.gpsimd.collective_compute(
                "AllGather", OP.bypass, ins=[agi2[:]], outs=[ago[:]],
                replica_groups=[list(range(8))])

            # ---------- ad2-gather burst (overlaps the AllGather) ----------
            with tc.tile_pool(name="pad2", bufs=1) as pd2:
                aix2 = pd2.tile([128, NSLOT // 16], i16)
                nc.sync.dma_start(aix2[:], adidx_d[:])
                for ci, (slot0, nids) in enumerate(plan["adcalls"]):
                    _gather_small(
                        nc.gpsimd,
                        at2_all[:, slot0 // 128:(slot0 + nids) // 128, :],
                        agi2[:, 40:42],
                        aix2[:, slot0 // 16:(slot0 + nids) // 16],
                        nids, 2, 128, queue_num=ci % 4)

            edge_pass(2)

    nc.finalize()
    return nc


def kernel(**inputs):
    per_core, plan = _host_prep(**inputs)
    if "nc" not in _CACHE:
        _CACHE["nc"] = _build_nc(plan)
    nc = _CACHE["nc"]
    from concourse.bass_utils import run_bass_kernel_spmd
    res = run_bass_kernel_spmd(nc, per_core, list(range(8)))
    full = np.concatenate([res.results[c]["out"] for c in range(8)], axis=0)
    return np.ascontiguousarray(full[:N]).astype(np.float32)
